# revision 17
# baseline (speedup 1.0000x reference)
"""Trainium2 Bass kernel for nn_HCMaps (RMSNorm + per-token hyper-coefficient maps).

Math (per token-stream vector x of length DIM=2048):
    u     = mean(x^2) + eps
    s     = rsqrt(u)                    (computed as exp(-0.5*ln u) on ACT)
    dot_j = (theta_j * rms_w) . x       (6 dots: pre, post, res[0..3])
    H_j   = alpha_j * tanh(s * dot_j) + B_j

Strategy:
  - Host pre-transposes x to [DIM, rows] per core so the contraction dim lands
    on SBUF partitions; the 6 dots become tiny-stationary fp32r matmuls.
  - sum(x^2) via ACT/DVE square + ones-stationary matmul, broadcast onto 6
    partitions for free by using a [128,6] ones stationary.
  - Everything stays in [6, rows] layout through the final fused alpha*tanh+B.
  - 8 cores data-parallel over rows; no collectives.
"""

import os
import sys
from contextlib import ExitStack

import numpy as np

if "/opt/trn_rl_repo" not in sys.path:
    sys.path.insert(0, "/opt/trn_rl_repo")

B, T, N, DIM = 2, 4096, 4, 2048
EPS = 1e-6
N_CORES = 8
ROWS = B * T * N            # 32768 total (b,t,n) rows
ROWS_PC = ROWS // N_CORES   # 4096 rows per core
TILE_R = 512                # rows per on-chip tile
N_TILES = ROWS_PC // TILE_R # 8
N_CH = DIM // 128           # 16 contraction chunks
DMA_GRP = 4                 # chunks per dma_start (1 MiB each)
NQ = 6                      # pre, post, res0..res3

LAST_RESULTS = None         # BassKernelResults of the most recent run (for test.py)

_CACHE = {}


def _build_program():
    """Build + compile the Bass program once; reuse across kernel() calls."""
    if "nc" in _CACHE:
        return _CACHE["nc"]

    import concourse.bacc as bacc
    import concourse.mybir as mybir
    import concourse.tile as tile

    f32 = mybir.dt.float32
    f32r = mybir.dt.float32r
    AF = mybir.ActivationFunctionType
    OP = mybir.AluOpType

    nc = bacc.Bacc("TRN2", target_bir_lowering=False, debug=False,
                   num_devices=N_CORES)

    NQ2 = 2 * NQ  # theta columns 0..5, ones columns 6..11 (ssq stationary)
    xt_d = nc.dram_tensor("xt", [DIM, ROWS_PC], f32r, kind="ExternalInput").ap()
    # host supplies theta pre-shuffled to SBUF chunk-major layout [128, 16*12]
    th_d = nc.dram_tensor("th", [128, N_CH * NQ2], f32r, kind="ExternalInput").ap()
    am_d = nc.dram_tensor("amat", [NQ, 1], f32, kind="ExternalInput").ap()
    bm_d = nc.dram_tensor("bmat", [NQ, TILE_R], f32, kind="ExternalInput").ap()
    out_d = nc.dram_tensor("out", [NQ, ROWS_PC], f32, kind="ExternalOutput").ap()

    with tile.TileContext(nc) as tc, ExitStack() as ctx:
        consts = ctx.enter_context(tc.tile_pool(name="consts", bufs=1))
        consts2 = ctx.enter_context(tc.tile_pool(name="consts2", bufs=1))
        consts3 = ctx.enter_context(tc.tile_pool(name="consts3", bufs=1))
        xpool = ctx.enter_context(tc.tile_pool(name="x", bufs=2))
        sqpool = ctx.enter_context(tc.tile_pool(name="sq", bufs=4))
        spool = ctx.enter_context(tc.tile_pool(name="small", bufs=3))
        hpool = ctx.enter_context(tc.tile_pool(name="h", bufs=1))
        psA = ctx.enter_context(tc.tile_pool(name="psA", bufs=2, space="PSUM"))
        psB = ctx.enter_context(tc.tile_pool(name="psB", bufs=2, space="PSUM"))

        # --- constants ---
        th_sb = consts.tile([128, N_CH * NQ2], f32r)  # [128, 192], chunk-major
        nc.sync.dma_start(out=th_sb[:], in_=th_d)
        am_sb = consts2.tile([NQ, 1], f32)
        nc.sync.dma_start(out=am_sb[:], in_=am_d)
        bm_sb = consts3.tile([NQ, TILE_R], f32)
        nc.sync.dma_start(out=bm_sb[:], in_=bm_d)

        h_all = hpool.tile([NQ, ROWS_PC], f32)       # pre-tanh args, all tiles
        o_sb = hpool.tile([NQ, ROWS_PC], f32)        # final outputs

        xt_r = xt_d.rearrange("(c p) r -> p c r", p=128)   # [128, 16, ROWS_PC]
        grp_r = DMA_GRP * TILE_R

        for t in range(N_TILES):
            r0 = t * TILE_R
            xt = xpool.tile([128, N_CH * TILE_R], f32r, tag="x")
            xt3 = xt[:].rearrange("p (c r) -> p c r", r=TILE_R)

            pa = psA.tile([NQ, TILE_R], f32, tag="psA")   # dots
            pb = psB.tile([NQ, TILE_R], f32, tag="psB")   # ssq (replicated x6)

            for g in range(N_CH // DMA_GRP):
                c0 = g * DMA_GRP
                nc.sync.dma_start(
                    out=xt3[:, c0:c0 + DMA_GRP],
                    in_=xt_r[:, c0:c0 + DMA_GRP, r0:r0 + TILE_R],
                )
                # squares for this group (split ACT/DVE to balance engines)
                sq = sqpool.tile([128, grp_r], f32r, tag="sq")
                xg = xt[:, c0 * TILE_R:(c0 + DMA_GRP) * TILE_R].bitcast(f32)
                if g % 2 == 0:
                    nc.scalar.activation(sq[:], xg, AF.Square)
                else:
                    nc.vector.tensor_tensor(out=sq[:], in0=xg, in1=xg, op=OP.mult)

                for i in range(DMA_GRP):
                    c = c0 + i
                    nc.tensor.matmul(
                        pa[:],
                        lhsT=th_sb[:, c * NQ2:c * NQ2 + NQ],
                        rhs=xt[:, c * TILE_R:(c + 1) * TILE_R],
                        start=(c == 0), stop=(c == N_CH - 1),
                    )
                    nc.tensor.matmul(
                        pb[:],
                        lhsT=th_sb[:, c * NQ2 + NQ:(c + 1) * NQ2],
                        rhs=sq[:, i * TILE_R:(i + 1) * TILE_R],
                        start=(c == 0), stop=(c == N_CH - 1),
                    )

            # u = ssq/DIM + eps ; s = exp(-0.5 ln u) = rsqrt(u) ; h = dots*s
            u_sb = spool.tile([NQ, TILE_R], f32, tag="u")
            nc.vector.tensor_scalar(
                out=u_sb[:], in0=pb[:], scalar1=1.0 / DIM, scalar2=EPS,
                op0=OP.mult, op1=OP.add,
            )
            ln_sb = spool.tile([NQ, TILE_R], f32, tag="ln")
            nc.scalar.activation(ln_sb[:], u_sb[:], AF.Ln)
            s_sb = spool.tile([NQ, TILE_R], f32, tag="s")
            nc.scalar.activation(s_sb[:], ln_sb[:], AF.Exp, scale=-0.5)
            nc.vector.tensor_tensor(
                out=h_all[:, r0:r0 + TILE_R], in0=pa[:], in1=s_sb[:], op=OP.mult,
            )

        # --- tail: tanh (one ACT table switch), fused alpha*th + B, store ---
        for t in range(N_TILES):
            r0 = t * TILE_R
            tt = spool.tile([NQ, TILE_R], f32, tag="tt")
            nc.scalar.activation(tt[:], h_all[:, r0:r0 + TILE_R], AF.Tanh)
            nc.vector.scalar_tensor_tensor(
                out=o_sb[:, r0:r0 + TILE_R], in0=tt[:], scalar=am_sb[:],
                in1=bm_sb[:], op0=OP.mult, op1=OP.add,
            )
        nc.sync.dma_start(out=out_d, in_=o_sb[:])

    nc.compile()
    _CACHE["nc"] = nc
    return nc


def _prep_inputs(x_stream, rms_weight, alpha_pre, alpha_post, alpha_res,
                 theta_pre, theta_post, theta_res, b_pre, b_post, b_res):
    x = np.asarray(x_stream, dtype=np.float32).reshape(ROWS, DIM)
    w = np.asarray(rms_weight, dtype=np.float32)
    th = np.ones((DIM, 2 * NQ), dtype=np.float32)
    th[:, 0] = np.asarray(theta_pre, np.float32) * w
    th[:, 1] = np.asarray(theta_post, np.float32) * w
    th[:, 2:NQ] = (np.asarray(theta_res, np.float32) * w[None, :]).T
    # shuffle to SBUF chunk-major layout: th_sb[p, c*12+j] = th[c*128+p, j]
    th = np.ascontiguousarray(
        th.reshape(N_CH, 128, 2 * NQ).transpose(1, 0, 2).reshape(128, N_CH * 2 * NQ))
    am = np.array([alpha_pre, alpha_post] + [alpha_res] * (NQ - 2),
                  dtype=np.float32).reshape(NQ, 1)
    bm = np.empty((NQ, TILE_R), dtype=np.float32)
    nidx = np.arange(TILE_R) % N
    bm[0] = np.asarray(b_pre, np.float32)[nidx]
    bm[1] = np.asarray(b_post, np.float32)[nidx]
    bm[2:] = np.asarray(b_res, np.float32)[:, nidx]
    in_maps = []
    for c in range(N_CORES):
        xt = np.ascontiguousarray(x[c * ROWS_PC:(c + 1) * ROWS_PC, :].T)
        in_maps.append({"xt": xt, "th": th, "amat": am, "bmat": bm})
    return in_maps


def _assemble(results):
    packed = np.stack([results[c]["out"] for c in range(N_CORES)])  # [8,6,4096]
    h_pre = packed[:, 0, :].reshape(B, T, N).astype(np.float32)
    h_post = packed[:, 1, :].reshape(B, T, N).astype(np.float32)
    hres = packed[:, 2:, :].transpose(0, 2, 1).reshape(B, T, N, N)
    h_res = np.ascontiguousarray(hres.transpose(0, 1, 3, 2))  # [b,t,i,n]
    return h_pre, h_post, h_res


def _install_ntff_hook_shim():
    """This image's antenv lacks axon_hooks; provide it so trace=True works."""
    import types
    try:
        import antenv.axon_hooks  # noqa: F401
        return
    except ImportError:
        pass
    mod = types.ModuleType("antenv.axon_hooks")
    mod._hook = None
    mod.set_axon_ntff_profile_hook = lambda h: setattr(mod, "_hook", h)
    mod.get_axon_ntff_profile_hook = lambda: mod._hook
    sys.modules["antenv.axon_hooks"] = mod
    try:
        import antenv
        antenv.axon_hooks = mod
        from trn_agent_boot.trn_boot import _ntff_profile_via_ctypes
        mod._hook = _ntff_profile_via_ctypes("/opt/axon/libaxon_pjrt.so")
    except Exception:
        mod._hook = None


def kernel(**inputs):
    global LAST_RESULTS
    if os.environ.get("BASS_TRACE"):
        _install_ntff_hook_shim()
    from concourse.bass_utils import run_bass_kernel_spmd

    nc = _build_program()
    in_maps = _prep_inputs(**inputs)
    trace = bool(os.environ.get("BASS_TRACE"))
    res = run_bass_kernel_spmd(nc, in_maps, core_ids=list(range(N_CORES)),
                               trace=trace)
    LAST_RESULTS = res
    return _assemble(res.results)
